# revision 15
# baseline (speedup 1.0000x reference)
"""DiSA (dimension-wise self-attention) Trainium2 kernel, v3.

Shapes (hardcoded): x [2, 256, 512], d_e = d_h = 512, tanh clip C = 5,
forward (i < j) causal mask, softmax over the dependent axis j.

Math: the reference computes, per (batch, head-row i, channel h),
softmax_j over tanh-clipped logits

    w(i,j,h) = exp(C tanh((dep[j,h] + head[i,h])/C)),   j > i.

v3 replaces the O(L^2 H) elementwise tanh/exp with an exponential-sum
approximation of g(s) = exp(C tanh(s/C)),

    g(s) ~= sum_k c_k exp(a_k s)  =>  w = sum_k c_k e^{a_k dep_j} e^{a_k head_i},

which is separable in (i, j).  The softmax numerator/denominator become
per-k SUFFIX sums over j of e^{a_k dep} (resp. * rep) — computed with
reverse tensor_tensor_scan on DVE in the natural [h-part, j-free]
layout, then combined as S = sum_k c_k psi_k * suf_k on DVE/Pool.  The
a_k are uniformly spaced so the exp tensors chain by one multiply:
E_{k+1} = E_k * E_delta.  Cancellation-sensitive arithmetic (E, psi,
scans, MAC) stays fp32; matmuls run fp32r (1 cyc/row, ~1.4e-4 rel).

Device-side value shifts (all exact, folded into host-made biases):
  repc  = elu(v) + 1 = min(exp(v),1) + max(v,0)   (2 ops instead of 4)
  attnc = attn + 1 = (suffix-sum of E*repc)/S     (shift cancels in d)
  out+1 is returned; the host subtracts 1 while assembling.

Sharding: 8 cores = (batch b) x (h-slice hs of 128 channels).  The gate
term attn@wf2 contracts the full h: each core computes the transposed
partial Z^T = wf2^T @ attnT-part (bf16) and a 4-core ReduceScatter(add)
hands every core its own h-slice rows.  Output stays [h-part, i-free];
the host transposes while assembling.  The fully-masked row i = 255
reduces (exactly, as in the reference) to a uniform average over j.

Measured end-to-end max relative error vs the exact reference: ~4.6e-3
(gate 2e-2).
"""

import numpy as np

B, L, DH = 2, 256, 512
P = 128
N_CORES = 8

# exp-sum fit of exp(5*tanh(s/5)) over s in [-7.7, 7.9]; uniform alphas
# so E_{k+1} = E_k * E_delta.
ALPHAS = [-0.15, 0.01428571428571429, 0.17857142857142858,
          0.34285714285714286, 0.5071428571428571, 0.6714285714285714,
          0.8357142857142857, 1.0]
CS = [
    -0.35870424433604337, 4.457071549512478, -22.137854870724844,
    56.39434504343734, -77.22572995183789, 54.336340442121426,
    -16.105790422879487, 1.6768402933274777,
]
NK = len(ALPHAS)
A0 = ALPHAS[0]
AD = ALPHAS[1] - ALPHAS[0]

_CACHE = {}


def _patch_concourse():
    """This environment's walrus accepts at most ONE sync-wait command per
    instruction; hoist all but the last wait of every instruction onto
    fresh same-engine NoOps placed directly before it."""
    import bass_rust as _br
    import concourse.bass as bass
    from concourse import mybir

    if getattr(bass.Bass, "_multiwait_patched", False):
        return
    orig_to_json_bytes = bass.Bass.to_json_bytes

    def _fix(self):
        n = 0
        for f in self.m.functions:
            for blk in f.blocks:
                out = []
                changed = False
                for inst in blk.instructions:
                    si = inst.sync_info
                    if si is not None and len(si.on_wait) > 1:
                        waits = list(si.on_wait)
                        for w in waits[:-1]:
                            nop = mybir.InstNoOp(
                                name=f"hoistw-{n}", ins=[], outs=[]
                            )
                            n += 1
                            nop.engine = inst.engine
                            nop.sync_info = _br.SyncInfo(
                                on_wait=[w], on_update=[]
                            )
                            out.append(nop)
                        inst.sync_info = _br.SyncInfo(
                            on_wait=[waits[-1]], on_update=list(si.on_update)
                        )
                        changed = True
                    out.append(inst)
                if changed:
                    blk.instructions = out

    def to_json_bytes(self):
        _fix(self)
        return orig_to_json_bytes(self)

    bass.Bass.to_json_bytes = to_json_bytes
    bass.Bass._multiwait_patched = True


def _rev(ap2d):
    """Reverse the (single) free dim of a 2D AP."""
    import concourse.bass as bass

    (pstride, pnum), (fstride, fnum) = ap2d.ap
    return bass.AP(
        tensor=ap2d.tensor,
        offset=ap2d.offset + fstride * (fnum - 1),
        ap=[[pstride, pnum], [-fstride, fnum]],
    )


def _bcast_mid(ap2d, nr):
    """[P, W] -> [P, nr, W] with a stride-0 broadcast middle dim."""
    import concourse.bass as bass

    return bass.AP(
        tensor=ap2d.tensor, offset=ap2d.offset,
        ap=[ap2d.ap[0], [0, nr], ap2d.ap[1]],
    )


def _build(repeat=1):
    if repeat in _CACHE:
        return _CACHE[repeat]
    _patch_concourse()
    import concourse.bass as bass
    import concourse.tile as tile
    from concourse import mybir

    F = mybir.ActivationFunctionType
    A = mybir.AluOpType
    f32 = mybir.dt.float32
    f32r = mybir.dt.float32r
    bf16 = mybir.dt.bfloat16

    nc = bass.Bass()
    xbt = nc.declare_dram_parameter("xbt", [DH, L], f32, isOutput=False)
    fcw = nc.declare_dram_parameter("fcw", [DH, DH], f32, isOutput=False)
    fcb = nc.declare_dram_parameter("fcb", [DH, 1], f32, isOutput=False)
    w12c = nc.declare_dram_parameter("w12c", [DH, 2 * P], f32, isOutput=False)
    wf1c = nc.declare_dram_parameter("wf1c", [DH, P], f32, isOutput=False)
    wf2r = nc.declare_dram_parameter("wf2r", [P, DH], bf16, isOutput=False)
    w1bc = nc.declare_dram_parameter("w1bc", [P, 1], f32, isOutput=False)
    hbc = nc.declare_dram_parameter("hbc", [P, 1], f32, isOutput=False)
    bfc = nc.declare_dram_parameter("bfc", [P, 1], f32, isOutput=False)
    out_hsT = nc.declare_dram_parameter("out_hsT", [P, L], f32, isOutput=True)

    zin = [nc.dram_tensor(f"zin{p}", [DH, L], bf16) for p in range(2)]
    zrs = [nc.dram_tensor(f"zrs{p}", [P, L], bf16) for p in range(2)]

    with tile.TileContext(nc) as tc:
        with (
            tc.tile_pool(name="consts", bufs=1) as consts,
            tc.tile_pool(name="wts", bufs=1) as wts,
            tc.tile_pool(name="st1", bufs=2) as st1,
            tc.tile_pool(name="ek", bufs=3) as ekp,
            tc.tile_pool(name="psi", bufs=3) as psip,
            tc.tile_pool(name="scn", bufs=3) as scn,
            tc.tile_pool(name="mac", bufs=2) as macp,
            tc.tile_pool(name="st3", bufs=2) as st3,
            tc.tile_pool(name="pm", bufs=2, space="PSUM") as pm,
            tc.tile_pool(name="pdh", bufs=1, space="PSUM") as pdh,
            tc.tile_pool(name="pz", bufs=1, space="PSUM") as pz,
        ):
            # ---- constants / weights (outside the repeat loop) ----------
            zeros = consts.tile([P, L], f32)
            nc.vector.memset(zeros[:], 0.0)
            fcb_col = consts.tile([P, 4], f32)
            nc.sync.dma_start(
                out=fcb_col[:],
                in_=fcb[:].rearrange("(t p) one -> p (t one)", p=P),
            )
            w1b_col = consts.tile([P, 1], f32)
            nc.sync.dma_start(out=w1b_col[:], in_=w1bc[:])
            hb_col = consts.tile([P, 1], f32)
            nc.sync.dma_start(out=hb_col[:], in_=hbc[:])
            bf_col = consts.tile([P, 1], f32)
            nc.sync.dma_start(out=bf_col[:], in_=bfc[:])
            # exp-chain biases: alpha*(bias) columns for E0/Edelta/psi0/psidelta
            eb = consts.tile([P, 4], f32)
            nc.vector.tensor_scalar_mul(out=eb[:, 0:1], in0=w1b_col[:], scalar1=A0)
            nc.vector.tensor_scalar_mul(out=eb[:, 1:2], in0=w1b_col[:], scalar1=AD)
            nc.vector.tensor_scalar_mul(out=eb[:, 2:3], in0=hb_col[:], scalar1=A0)
            nc.vector.tensor_scalar_mul(out=eb[:, 3:4], in0=hb_col[:], scalar1=AD)

            # weights: DMA fp32 then one-time round to fp32r for the PE
            fcw_t, w12_t, wf1_t = [], [], []
            for kt in range(4):
                sl = slice(kt * P, (kt + 1) * P)
                t = wts.tile([P, DH], f32, tag=f"fcwL{kt}", name=f"fcwL{kt}")
                nc.sync.dma_start(out=t[:], in_=fcw[sl, :])
                tr = wts.tile([P, DH], f32r, tag=f"fcw{kt}", name=f"fcwr{kt}")
                nc.scalar.copy(out=tr[:], in_=t[:])
                fcw_t.append(tr)
                t = wts.tile([P, 2 * P], f32, tag=f"w12L{kt}", name=f"w12L{kt}")
                nc.sync.dma_start(out=t[:], in_=w12c[sl, :])
                tr = wts.tile([P, 2 * P], f32r, tag=f"w12{kt}", name=f"w12r{kt}")
                nc.scalar.copy(out=tr[:], in_=t[:])
                w12_t.append(tr)
                t = wts.tile([P, P], f32, tag=f"wf1L{kt}", name=f"wf1L{kt}")
                nc.sync.dma_start(out=t[:], in_=wf1c[sl, :])
                tr = wts.tile([P, P], f32r, tag=f"wf1{kt}", name=f"wf1r{kt}")
                nc.scalar.copy(out=tr[:], in_=t[:])
                wf1_t.append(tr)
            wf2_sb = wts.tile([P, DH], bf16)
            nc.sync.dma_start(out=wf2_sb[:], in_=wf2r[:])

            xT = []
            for dt in range(4):
                t = wts.tile([P, L], f32, tag=f"xTL{dt}", name=f"xTL{dt}")
                nc.sync.dma_start(out=t[:], in_=xbt[dt * P:(dt + 1) * P, :])
                tr = wts.tile([P, L], f32r, tag=f"xT{dt}", name=f"xT{dt}")
                nc.scalar.copy(out=tr[:], in_=t[:])
                xT.append(tr)

            for _rep in range(repeat):
                # ---- stage 1: rep_map^T (+1 shifted), dep/head ----------
                # repc = elu(v) + 1 = min(exp(v), 1) + max(v, 0)
                repT = [st1.tile([P, L], f32r, tag=f"repT{ht}", name=f"repT{ht}")
                        for ht in range(4)]
                for ht in range(4):
                    ps = pm.tile([P, L], f32)
                    for dt in range(4):
                        nc.tensor.matmul(
                            out=ps[:],
                            lhsT=fcw_t[dt][:, ht * P:(ht + 1) * P],
                            rhs=xT[dt][:],
                            start=(dt == 0),
                            stop=(dt == 3),
                        )
                    pos = st1.tile([P, L], f32, tag="epos")
                    e = st1.tile([P, L], f32, tag="eexp")
                    bias = fcb_col[:, ht:ht + 1]
                    nc.scalar.activation(
                        out=pos[:], in_=ps[:], func=F.Relu, bias=bias, scale=1.0,
                    )
                    nc.scalar.activation(
                        out=e[:], in_=ps[:], func=F.Exp, bias=bias, scale=1.0,
                    )
                    nc.vector.scalar_tensor_tensor(
                        out=repT[ht][:], in0=e[:], scalar=1.0, in1=pos[:],
                        op0=A.min, op1=A.add,
                    )

                rep_hs = repT[0][:].bitcast(f32)  # own h-slice (+1 shifted)

                psd = pdh.tile([P, L], f32, tag="psd", name="psd")
                psh = pdh.tile([P, L], f32, tag="psh", name="psh")
                for et in range(4):
                    nc.tensor.matmul(
                        out=psd[:], lhsT=w12_t[et][:, 0:P], rhs=repT[et][:],
                        start=(et == 0), stop=(et == 3),
                    )
                for et in range(4):
                    nc.tensor.matmul(
                        out=psh[:], lhsT=w12_t[et][:, P:2 * P], rhs=repT[et][:],
                        start=(et == 0), stop=(et == 3),
                    )

                # gate left term: zl = wf1^T @ repc + bf'  (bias folds the
                # repc/attnc shifts and the wf2 column sums)
                psz = pz.tile([P, L], f32, tag="psz", name="psz")
                for ht in range(4):
                    nc.tensor.matmul(
                        out=psz[:], lhsT=wf1_t[ht][:], rhs=repT[ht][:],
                        start=(ht == 0), stop=(ht == 3),
                    )
                zl = st3.tile([P, L], f32, tag="zl")
                nc.scalar.activation(
                    out=zl[:], in_=psz[:], func=F.Identity,
                    bias=bf_col[:], scale=1.0,
                )

                # ---- stage 2: exp-sum suffix attention ------------------
                # E/psi seeds and deltas read dep/head straight from PSUM.
                Ed = st1.tile([P, L], f32, tag="Ed")
                Pd = st1.tile([P, L], f32, tag="Pd")
                nc.scalar.activation(
                    out=Ed[:], in_=psd[:], func=F.Exp,
                    bias=eb[:, 1:2], scale=AD,
                )
                nc.scalar.activation(
                    out=Pd[:], in_=psh[:], func=F.Exp,
                    bias=eb[:, 3:4], scale=AD,
                )

                accS = macp.tile([P, L - 1], f32, tag="accS", name="accS")
                accA = macp.tile([P, L - 1], f32, tag="accA", name="accA")
                Ek_prev = None
                Ps_prev = None
                for k in range(NK):
                    Ek = ekp.tile([P, L], f32, tag=f"E{k % 3}", name=f"E{k}")
                    Ps = psip.tile([P, L], f32, tag=f"P{k % 3}", name=f"P{k}")
                    if k == 0:
                        nc.scalar.activation(
                            out=Ek[:], in_=psd[:], func=F.Exp,
                            bias=eb[:, 0:1], scale=A0,
                        )
                        nc.scalar.activation(
                            out=Ps[:], in_=psh[:], func=F.Exp,
                            bias=eb[:, 2:3], scale=A0,
                        )
                    else:
                        nc.gpsimd.tensor_tensor(
                            out=Ek[:], in0=Ek_prev[:], in1=Ed[:], op=A.mult
                        )
                        nc.gpsimd.tensor_tensor(
                            out=Ps[:], in0=Ps_prev[:], in1=Pd[:], op=A.mult
                        )
                    Ek_prev, Ps_prev = Ek, Ps

                    EkR = ekp.tile([P, L], f32, tag=f"ER{k % 3}", name=f"ER{k}")
                    nc.gpsimd.tensor_tensor(
                        out=EkR[:], in0=Ek[:], in1=rep_hs, op=A.mult
                    )
                    # reverse inclusive scans: ssc[:,q,j] = sum_{j'>=j}
                    ssc = scn.tile([P, 2, L], f32, tag=f"ssc{k % 3}", name=f"ssc{k}")
                    sS = ssc[:, 0, :]
                    sA = ssc[:, 1, :]
                    nc.vector.tensor_tensor_scan(
                        out=_rev(sS), data0=_rev(Ek[:]), data1=_rev(Ek[:]),
                        initial=0.0, op0=A.add, op1=A.bypass,
                    )
                    nc.vector.tensor_tensor_scan(
                        out=_rev(sA), data0=_rev(EkR[:]), data1=_rev(EkR[:]),
                        initial=0.0, op0=A.add, op1=A.bypass,
                    )
                    # MAC: acc{S,A} (+)= c_k * psi * suffix   (i = 0..254)
                    t2 = macp.tile([P, 2, L - 1], f32, tag=f"t{k % 2}", name=f"t{k}")
                    nc.vector.scalar_tensor_tensor(
                        out=t2[:], in0=ssc[:, :, 1:L], scalar=float(CS[k]),
                        in1=_bcast_mid(Ps[:, 0:L - 1], 2),
                        op0=A.mult, op1=A.mult,
                    )
                    if k == 0:
                        nc.gpsimd.tensor_tensor(
                            out=accS[:], in0=t2[:, 0, :],
                            in1=zeros[:, 0:L - 1], op=A.add,
                        )
                        nc.gpsimd.tensor_tensor(
                            out=accA[:], in0=t2[:, 1, :],
                            in1=zeros[:, 0:L - 1], op=A.add,
                        )
                    else:
                        nc.gpsimd.tensor_tensor(
                            out=accS[:], in0=t2[:, 0, :], in1=accS[:], op=A.add,
                        )
                        nc.gpsimd.tensor_tensor(
                            out=accA[:], in0=t2[:, 1, :], in1=accA[:], op=A.add,
                        )

                # attnc = attn + 1 = accA/accS (repc shift); row 255 uniform
                attnT = st1.tile([P, L], f32, tag="attnT", name="attnT")
                rS = st1.tile([P, L - 1], f32, tag="rS")
                nc.vector.reciprocal(out=rS[:], in_=accS[:])
                nc.vector.tensor_tensor(
                    out=attnT[:, 0:L - 1], in0=accA[:], in1=rS[:], op=A.mult
                )
                nc.vector.tensor_reduce(
                    out=attnT[:, L - 1:L], in_=rep_hs,
                    axis=mybir.AxisListType.X, op=A.add,
                )
                nc.vector.tensor_scalar_mul(
                    out=attnT[:, L - 1:L], in0=attnT[:, L - 1:L],
                    scalar1=1.0 / L,
                )

                # ---- stage 3: Z^T partials, ReduceScatter, gate, mix ----
                attn_b = st3.tile([P, L], bf16, tag="attnb")
                nc.scalar.copy(out=attn_b[:], in_=attnT[:])
                zi = zin[_rep % 2]
                zr = zrs[_rep % 2]
                for kt in range(4):
                    ps = pm.tile([P, L], f32)
                    nc.tensor.matmul(
                        out=ps[:],
                        lhsT=wf2_sb[:, kt * P:(kt + 1) * P],
                        rhs=attn_b[:], start=True, stop=True,
                    )
                    zs = st3.tile([P, L], bf16, tag=f"zin{kt}")
                    nc.scalar.copy(out=zs[:], in_=ps[:])
                    nc.sync.dma_start(
                        out=zi[kt * P:(kt + 1) * P, :], in_=zs[:]
                    )
                nc.gpsimd.collective_compute(
                    "ReduceScatter", A.add,
                    replica_groups=[[0, 1, 2, 3], [4, 5, 6, 7]],
                    ins=[zi[:]], outs=[zr[:]],
                )
                zo = st3.tile([P, L], bf16, tag="zo")
                nc.sync.dma_start(out=zo[:], in_=zr[:])
                # gate via tanh (same act table as exp):
                #   sigmoid(t) = 0.5*tanh(t/2) + 0.5
                # out+1 = attnc + 0.5*(d + gt*d),  d = repc - attnc
                t3 = st3.tile([P, L], f32, tag="t3")
                nc.vector.tensor_tensor(out=t3[:], in0=zl[:], in1=zo[:], op=A.add)
                gt = st3.tile([P, L], f32, tag="gt")
                nc.scalar.activation(out=gt[:], in_=t3[:], func=F.Tanh, scale=0.5)
                d = st3.tile([P, L], f32, tag="d")
                nc.gpsimd.tensor_tensor(
                    out=d[:], in0=rep_hs, in1=attnT[:], op=A.subtract
                )
                m = st3.tile([P, L], f32, tag="m")
                nc.gpsimd.tensor_tensor(out=m[:], in0=gt[:], in1=d[:], op=A.mult)
                u = st3.tile([P, L], f32, tag="u")
                nc.gpsimd.tensor_tensor(out=u[:], in0=d[:], in1=m[:], op=A.add)
                o = st3.tile([P, L], f32, tag="o")
                nc.vector.scalar_tensor_tensor(
                    out=o[:], in0=u[:], scalar=0.5, in1=attnT[:],
                    op0=A.mult, op1=A.add,
                )
                nc.sync.dma_start(out=out_hsT[:], in_=o[:])

    _CACHE[repeat] = nc
    return nc


def _make_in_maps(inputs):
    import ml_dtypes

    nbf = ml_dtypes.bfloat16
    x = np.asarray(inputs["x"], np.float32)
    fc_w = np.ascontiguousarray(np.asarray(inputs["fc_w"], np.float32))
    fc_b = np.asarray(inputs["fc_b"], np.float32)
    w1_w = np.asarray(inputs["w1_w"], np.float32)
    w1_b = np.asarray(inputs["w1_b"], np.float32)
    w2_w = np.asarray(inputs["w2_w"], np.float32)
    w2_b = np.asarray(inputs["w2_b"], np.float32)
    b_logit = np.asarray(inputs["b_logit"], np.float32)
    wf1_w = np.asarray(inputs["wf1_w"], np.float32)
    wf2_w = np.asarray(inputs["wf2_w"], np.float32)
    bf = np.asarray(inputs["bf"], np.float32)

    # device works with repc = rep+1 and attnc = attn+1; fold the -1
    # shifts into the matmul biases (rep @ W = repc @ W - colsum(W)).
    w1_cs = w1_w.sum(axis=0)
    w2_cs = w2_w.sum(axis=0)
    wf1_cs = wf1_w.sum(axis=0)
    wf2_cs = wf2_w.sum(axis=0)

    in_maps = []
    for c in range(N_CORES):
        b, hs = c // 4, c % 4
        H = slice(P * hs, P * (hs + 1))
        # fcw column order: ht=0 must be the core's own h-slice so that
        # repT[0] is rep_hs; remaining slices fill ht=1..3.
        order = [hs] + [t for t in range(4) if t != hs]
        fcw_perm = np.ascontiguousarray(
            np.concatenate([fc_w[:, P * t:P * (t + 1)] for t in order], axis=1)
        )
        fcb_perm = np.ascontiguousarray(
            np.concatenate([fc_b[P * t:P * (t + 1)] for t in order])
        ).reshape(DH, 1)
        # dep/head/wf1 matmuls contract over the permuted h order, so
        # their weight ROWS are permuted identically.
        rperm = np.concatenate(
            [np.arange(P * t, P * (t + 1)) for t in order]
        )
        in_maps.append({
            "xbt": np.ascontiguousarray(x[b].T),
            "fcw": fcw_perm,
            "fcb": fcb_perm,
            "w12c": np.ascontiguousarray(
                np.concatenate([w1_w[rperm][:, H], w2_w[rperm][:, H]], axis=1)
            ),
            "wf1c": np.ascontiguousarray(wf1_w[rperm][:, H]),
            "wf2r": np.ascontiguousarray(wf2_w[H, :].astype(nbf)),
            "w1bc": np.ascontiguousarray(
                (w1_b - w1_cs)[H].reshape(P, 1)
            ),
            "hbc": np.ascontiguousarray(
                (w2_b + b_logit - w2_cs)[H].reshape(P, 1)
            ),
            "bfc": np.ascontiguousarray(
                (bf - wf1_cs - wf2_cs)[H].reshape(P, 1)
            ),
        })
    return in_maps


def kernel(**inputs):
    from concourse.bass_utils import run_bass_kernel_spmd

    nc = _build()
    in_maps = _make_in_maps(inputs)
    res = run_bass_kernel_spmd(nc, in_maps, core_ids=list(range(N_CORES)))
    out = np.empty((B, L, DH), np.float32)
    for c in range(N_CORES):
        b, hs = c // 4, c % 4
        # device returns out+1 (attnc/repc shift); undo it here
        out[b, :, P * hs:P * (hs + 1)] = res.results[c]["out_hsT"].T - 1.0
    return out


# revision 17
# speedup vs baseline: 1.3142x; 1.3142x over previous
"""DiSA (dimension-wise self-attention) Trainium2 kernel, v3.

Shapes (hardcoded): x [2, 256, 512], d_e = d_h = 512, tanh clip C = 5,
forward (i < j) causal mask, softmax over the dependent axis j.

Math: the reference computes, per (batch, head-row i, channel h),
softmax_j over tanh-clipped logits

    w(i,j,h) = exp(C tanh((dep[j,h] + head[i,h])/C)),   j > i.

v3 replaces the O(L^2 H) elementwise tanh/exp with an exponential-sum
approximation of g(s) = exp(C tanh(s/C)),

    g(s) ~= sum_k c_k exp(a_k s)  =>  w = sum_k c_k e^{a_k dep_j} e^{a_k head_i},

which is separable in (i, j).  The softmax numerator/denominator become
per-k SUFFIX sums over j of e^{a_k dep} (resp. * rep) — computed with
reverse tensor_tensor_scan on DVE in the natural [h-part, j-free]
layout, then combined as S = sum_k c_k psi_k * suf_k on DVE/Pool.  The
a_k are uniformly spaced so the exp tensors chain by one multiply:
E_{k+1} = E_k * E_delta.  Cancellation-sensitive arithmetic (E, psi,
scans, MAC) stays fp32; matmuls run fp32r (1 cyc/row, ~1.4e-4 rel).

Device-side value shifts (all exact, folded into host-made biases):
  repc  = elu(v) + 1 = min(exp(v),1) + max(v,0)   (2 ops instead of 4)
  attnc = attn + 1 = (suffix-sum of E*repc)/S     (shift cancels in d)
  out+1 is returned; the host subtracts 1 while assembling.

Sharding: 8 cores = (batch b) x (h-slice hs of 128 channels).  The gate
term attn@wf2 contracts the full h: each core computes the transposed
partial Z^T = wf2^T @ attnT-part (bf16) and a 4-core ReduceScatter(add)
hands every core its own h-slice rows.  Output stays [h-part, i-free];
the host transposes while assembling.  The fully-masked row i = 255
reduces (exactly, as in the reference) to a uniform average over j.

Measured end-to-end max relative error vs the exact reference: ~4.6e-3
(gate 2e-2).
"""

import numpy as np

B, L, DH = 2, 256, 512
P = 128
N_CORES = 8

# exp-sum fit of exp(5*tanh(s/5)) over s in [-7.7, 7.9]; uniform alphas
# so E_{k+1} = E_k * E_delta.
ALPHAS = [-0.15, 0.01428571428571429, 0.17857142857142858,
          0.34285714285714286, 0.5071428571428571, 0.6714285714285714,
          0.8357142857142857, 1.0]
CS = [
    -0.35870424433604337, 4.457071549512478, -22.137854870724844,
    56.39434504343734, -77.22572995183789, 54.336340442121426,
    -16.105790422879487, 1.6768402933274777,
]
NK = len(ALPHAS)
A0 = ALPHAS[0]
AD = ALPHAS[1] - ALPHAS[0]

_CACHE = {}


def _patch_concourse():
    """This environment's walrus accepts at most ONE sync-wait command per
    instruction; hoist all but the last wait of every instruction onto
    fresh same-engine NoOps placed directly before it."""
    import bass_rust as _br
    import concourse.bass as bass
    from concourse import mybir

    if getattr(bass.Bass, "_multiwait_patched", False):
        return
    orig_to_json_bytes = bass.Bass.to_json_bytes

    def _fix(self):
        n = 0
        for f in self.m.functions:
            for blk in f.blocks:
                out = []
                changed = False
                for inst in blk.instructions:
                    si = inst.sync_info
                    if si is not None and len(si.on_wait) > 1:
                        waits = list(si.on_wait)
                        for w in waits[:-1]:
                            nop = mybir.InstNoOp(
                                name=f"hoistw-{n}", ins=[], outs=[]
                            )
                            n += 1
                            nop.engine = inst.engine
                            nop.sync_info = _br.SyncInfo(
                                on_wait=[w], on_update=[]
                            )
                            out.append(nop)
                        inst.sync_info = _br.SyncInfo(
                            on_wait=[waits[-1]], on_update=list(si.on_update)
                        )
                        changed = True
                    out.append(inst)
                if changed:
                    blk.instructions = out

    def to_json_bytes(self):
        _fix(self)
        return orig_to_json_bytes(self)

    bass.Bass.to_json_bytes = to_json_bytes
    bass.Bass._multiwait_patched = True


def _rev(ap2d):
    """Reverse the (single) free dim of a 2D AP."""
    import concourse.bass as bass

    (pstride, pnum), (fstride, fnum) = ap2d.ap
    return bass.AP(
        tensor=ap2d.tensor,
        offset=ap2d.offset + fstride * (fnum - 1),
        ap=[[pstride, pnum], [-fstride, fnum]],
    )


def _bcast_mid(ap2d, nr):
    """[P, W] -> [P, nr, W] with a stride-0 broadcast middle dim."""
    import concourse.bass as bass

    return bass.AP(
        tensor=ap2d.tensor, offset=ap2d.offset,
        ap=[ap2d.ap[0], [0, nr], ap2d.ap[1]],
    )


def _build(repeat=1):
    if repeat in _CACHE:
        return _CACHE[repeat]
    _patch_concourse()
    import concourse.bass as bass
    import concourse.tile as tile
    from concourse import mybir

    F = mybir.ActivationFunctionType
    A = mybir.AluOpType
    f32 = mybir.dt.float32
    f32r = mybir.dt.float32r
    bf16 = mybir.dt.bfloat16

    nc = bass.Bass()
    xbt = nc.declare_dram_parameter("xbt", [DH, L], f32, isOutput=False)
    fcw = nc.declare_dram_parameter("fcw", [DH, DH], f32, isOutput=False)
    fcb = nc.declare_dram_parameter("fcb", [DH, 1], f32, isOutput=False)
    w12c = nc.declare_dram_parameter("w12c", [DH, 2 * P], f32, isOutput=False)
    wf1c = nc.declare_dram_parameter("wf1c", [DH, P], f32, isOutput=False)
    wf2r = nc.declare_dram_parameter("wf2r", [P, DH], bf16, isOutput=False)
    w1bc = nc.declare_dram_parameter("w1bc", [P, 1], f32, isOutput=False)
    hbc = nc.declare_dram_parameter("hbc", [P, 1], f32, isOutput=False)
    bfc = nc.declare_dram_parameter("bfc", [P, 1], f32, isOutput=False)
    out_hsT = nc.declare_dram_parameter("out_hsT", [P, L], f32, isOutput=True)

    zin = [nc.dram_tensor(f"zin{p}", [DH, L], bf16) for p in range(2)]
    zrs = [nc.dram_tensor(f"zrs{p}", [P, L], bf16) for p in range(2)]

    with tile.TileContext(nc) as tc:
        with (
            tc.tile_pool(name="consts", bufs=1) as consts,
            tc.tile_pool(name="wts", bufs=1) as wts,
            tc.tile_pool(name="st1", bufs=2) as st1,
            tc.tile_pool(name="ek", bufs=3) as ekp,
            tc.tile_pool(name="psi", bufs=3) as psip,
            tc.tile_pool(name="scn", bufs=3) as scn,
            tc.tile_pool(name="mac", bufs=2) as macp,
            tc.tile_pool(name="st3", bufs=2) as st3,
            tc.tile_pool(name="pm", bufs=2, space="PSUM") as pm,
            tc.tile_pool(name="pdh", bufs=1, space="PSUM") as pdh,
            tc.tile_pool(name="pz", bufs=1, space="PSUM") as pz,
        ):
            # ---- constants / weights (outside the repeat loop) ----------
            zeros = consts.tile([P, L], f32)
            nc.vector.memset(zeros[:], 0.0)
            fcb_col = consts.tile([P, 4], f32)
            nc.sync.dma_start(
                out=fcb_col[:],
                in_=fcb[:].rearrange("(t p) one -> p (t one)", p=P),
            )
            w1b_col = consts.tile([P, 1], f32)
            nc.sync.dma_start(out=w1b_col[:], in_=w1bc[:])
            hb_col = consts.tile([P, 1], f32)
            nc.sync.dma_start(out=hb_col[:], in_=hbc[:])
            bf_col = consts.tile([P, 1], f32)
            nc.sync.dma_start(out=bf_col[:], in_=bfc[:])
            # exp-chain biases: alpha*(bias) columns for E0/Edelta/psi0/psidelta
            eb = consts.tile([P, 4], f32)
            nc.vector.tensor_scalar_mul(out=eb[:, 0:1], in0=w1b_col[:], scalar1=A0)
            nc.vector.tensor_scalar_mul(out=eb[:, 1:2], in0=w1b_col[:], scalar1=AD)
            nc.vector.tensor_scalar_mul(out=eb[:, 2:3], in0=hb_col[:], scalar1=A0)
            nc.vector.tensor_scalar_mul(out=eb[:, 3:4], in0=hb_col[:], scalar1=AD)

            # weights: DMA fp32 then one-time round to fp32r for the PE
            fcw_t, w12_t, wf1_t = [], [], []
            for kt in range(4):
                sl = slice(kt * P, (kt + 1) * P)
                t = wts.tile([P, DH], f32, tag=f"fcwL{kt}", name=f"fcwL{kt}")
                nc.sync.dma_start(out=t[:], in_=fcw[sl, :])
                tr = wts.tile([P, DH], f32r, tag=f"fcw{kt}", name=f"fcwr{kt}")
                nc.scalar.copy(out=tr[:], in_=t[:])
                fcw_t.append(tr)
                t = wts.tile([P, 2 * P], f32, tag=f"w12L{kt}", name=f"w12L{kt}")
                nc.sync.dma_start(out=t[:], in_=w12c[sl, :])
                tr = wts.tile([P, 2 * P], f32r, tag=f"w12{kt}", name=f"w12r{kt}")
                nc.scalar.copy(out=tr[:], in_=t[:])
                w12_t.append(tr)
                t = wts.tile([P, P], f32, tag=f"wf1L{kt}", name=f"wf1L{kt}")
                nc.sync.dma_start(out=t[:], in_=wf1c[sl, :])
                tr = wts.tile([P, P], f32r, tag=f"wf1{kt}", name=f"wf1r{kt}")
                nc.scalar.copy(out=tr[:], in_=t[:])
                wf1_t.append(tr)
            wf2_sb = wts.tile([P, DH], bf16)
            nc.sync.dma_start(out=wf2_sb[:], in_=wf2r[:])

            xT = []
            for dt in range(4):
                t = wts.tile([P, L], f32, tag=f"xTL{dt}", name=f"xTL{dt}")
                nc.sync.dma_start(out=t[:], in_=xbt[dt * P:(dt + 1) * P, :])
                tr = wts.tile([P, L], f32r, tag=f"xT{dt}", name=f"xT{dt}")
                nc.scalar.copy(out=tr[:], in_=t[:])
                xT.append(tr)

            for _rep in range(repeat):
                # ---- stage 1: rep_map^T (+1 shifted), dep/head ----------
                # repc = elu(v) + 1 = min(exp(v), 1) + max(v, 0)
                repT = [st1.tile([P, L], f32r, tag=f"repT{ht}", name=f"repT{ht}")
                        for ht in range(4)]
                for ht in range(4):
                    ps = pm.tile([P, L], f32)
                    for dt in range(4):
                        nc.tensor.matmul(
                            out=ps[:],
                            lhsT=fcw_t[dt][:, ht * P:(ht + 1) * P],
                            rhs=xT[dt][:],
                            start=(dt == 0),
                            stop=(dt == 3),
                        )
                    pos = st1.tile([P, L], f32, tag="epos")
                    e = st1.tile([P, L], f32, tag="eexp")
                    bias = fcb_col[:, ht:ht + 1]
                    nc.scalar.activation(
                        out=pos[:], in_=ps[:], func=F.Relu, bias=bias, scale=1.0,
                    )
                    nc.scalar.activation(
                        out=e[:], in_=ps[:], func=F.Exp, bias=bias, scale=1.0,
                    )
                    nc.vector.scalar_tensor_tensor(
                        out=repT[ht][:], in0=e[:], scalar=1.0, in1=pos[:],
                        op0=A.min, op1=A.add,
                    )

                rep_hs = repT[0][:].bitcast(f32)  # own h-slice (+1 shifted)

                psd = pdh.tile([P, L], f32, tag="psd", name="psd")
                psh = pdh.tile([P, L], f32, tag="psh", name="psh")
                for et in range(4):
                    nc.tensor.matmul(
                        out=psd[:], lhsT=w12_t[et][:, 0:P], rhs=repT[et][:],
                        start=(et == 0), stop=(et == 3),
                    )
                for et in range(4):
                    nc.tensor.matmul(
                        out=psh[:], lhsT=w12_t[et][:, P:2 * P], rhs=repT[et][:],
                        start=(et == 0), stop=(et == 3),
                    )

                # gate left term: zl = wf1^T @ repc + bf'  (bias folds the
                # repc/attnc shifts and the wf2 column sums)
                psz = pz.tile([P, L], f32, tag="psz", name="psz")
                for ht in range(4):
                    nc.tensor.matmul(
                        out=psz[:], lhsT=wf1_t[ht][:], rhs=repT[ht][:],
                        start=(ht == 0), stop=(ht == 3),
                    )
                zl = st3.tile([P, L], f32, tag="zl")
                nc.scalar.activation(
                    out=zl[:], in_=psz[:], func=F.Identity,
                    bias=bf_col[:], scale=1.0,
                )

                # ---- stage 2: exp-sum suffix attention ------------------
                # E/psi seeds and deltas read dep/head straight from PSUM.
                Ed = st1.tile([P, L], f32, tag="Ed")
                Pd = st1.tile([P, L], f32, tag="Pd")
                nc.scalar.activation(
                    out=Ed[:], in_=psd[:], func=F.Exp,
                    bias=eb[:, 1:2], scale=AD,
                )
                nc.scalar.activation(
                    out=Pd[:], in_=psh[:], func=F.Exp,
                    bias=eb[:, 3:4], scale=AD,
                )

                accS = macp.tile([P, L - 1], f32, tag="accS", name="accS")
                accA = macp.tile([P, L - 1], f32, tag="accA", name="accA")
                Ek_prev = None
                Ps_prev = None
                for k in range(NK):
                    Ek = ekp.tile([P, L], f32, tag=f"E{k % 3}", name=f"E{k}")
                    Ps = psip.tile([P, L], f32, tag=f"P{k % 3}", name=f"P{k}")
                    if k == 0:
                        nc.scalar.activation(
                            out=Ek[:], in_=psd[:], func=F.Exp,
                            bias=eb[:, 0:1], scale=A0,
                        )
                        nc.scalar.activation(
                            out=Ps[:], in_=psh[:], func=F.Exp,
                            bias=eb[:, 2:3], scale=A0,
                        )
                    else:
                        nc.gpsimd.tensor_tensor(
                            out=Ek[:], in0=Ek_prev[:], in1=Ed[:], op=A.mult
                        )
                        nc.gpsimd.tensor_tensor(
                            out=Ps[:], in0=Ps_prev[:], in1=Pd[:], op=A.mult
                        )
                    Ek_prev, Ps_prev = Ek, Ps

                    EkR = ekp.tile([P, L], f32, tag=f"ER{k % 3}", name=f"ER{k}")
                    nc.gpsimd.tensor_tensor(
                        out=EkR[:], in0=Ek[:], in1=rep_hs, op=A.mult
                    )
                    # reverse inclusive scans: ssc[:,q,j] = sum_{j'>=j}
                    ssc = scn.tile([P, 2, L], f32, tag=f"ssc{k % 3}", name=f"ssc{k}")
                    sS = ssc[:, 0, :]
                    sA = ssc[:, 1, :]
                    nc.vector.tensor_tensor_scan(
                        out=_rev(sS), data0=_rev(Ek[:]), data1=_rev(Ek[:]),
                        initial=0.0, op0=A.add, op1=A.bypass,
                    )
                    nc.vector.tensor_tensor_scan(
                        out=_rev(sA), data0=_rev(EkR[:]), data1=_rev(EkR[:]),
                        initial=0.0, op0=A.add, op1=A.bypass,
                    )
                    # MAC: acc{S,A} (+)= c_k * psi * suffix   (i = 0..254)
                    t2 = macp.tile([P, 2, L - 1], f32, tag=f"t{k % 2}", name=f"t{k}")
                    nc.vector.scalar_tensor_tensor(
                        out=t2[:], in0=ssc[:, :, 1:L], scalar=float(CS[k]),
                        in1=_bcast_mid(Ps[:, 0:L - 1], 2),
                        op0=A.mult, op1=A.mult,
                    )
                    if k == 0:
                        nc.gpsimd.tensor_tensor(
                            out=accS[:], in0=t2[:, 0, :],
                            in1=zeros[:, 0:L - 1], op=A.add,
                        )
                        nc.gpsimd.tensor_tensor(
                            out=accA[:], in0=t2[:, 1, :],
                            in1=zeros[:, 0:L - 1], op=A.add,
                        )
                    else:
                        nc.gpsimd.tensor_tensor(
                            out=accS[:], in0=t2[:, 0, :], in1=accS[:], op=A.add,
                        )
                        nc.gpsimd.tensor_tensor(
                            out=accA[:], in0=t2[:, 1, :], in1=accA[:], op=A.add,
                        )

                # attnc = attn + 1 = accA/accS (repc shift); row 255 uniform
                attnT = st1.tile([P, L], f32, tag="attnT", name="attnT")
                rS = st1.tile([P, L - 1], f32, tag="rS")
                nc.vector.reciprocal(out=rS[:], in_=accS[:])
                nc.vector.tensor_tensor(
                    out=attnT[:, 0:L - 1], in0=accA[:], in1=rS[:], op=A.mult
                )
                nc.vector.tensor_reduce(
                    out=attnT[:, L - 1:L], in_=rep_hs,
                    axis=mybir.AxisListType.X, op=A.add,
                )
                nc.vector.tensor_scalar_mul(
                    out=attnT[:, L - 1:L], in0=attnT[:, L - 1:L],
                    scalar1=1.0 / L,
                )

                # ---- stage 3: Z^T partials, ReduceScatter, gate, mix ----
                attn_b = st3.tile([P, L], bf16, tag="attnb")
                nc.scalar.copy(out=attn_b[:], in_=attnT[:])
                zi = zin[_rep % 2]
                zr = zrs[_rep % 2]
                for kt in range(4):
                    ps = pm.tile([P, L], f32)
                    nc.tensor.matmul(
                        out=ps[:],
                        lhsT=wf2_sb[:, kt * P:(kt + 1) * P],
                        rhs=attn_b[:], start=True, stop=True,
                    )
                    zs = st3.tile([P, L], bf16, tag=f"zin{kt}")
                    nc.scalar.copy(out=zs[:], in_=ps[:])
                    nc.sync.dma_start(
                        out=zi[kt * P:(kt + 1) * P, :], in_=zs[:]
                    )
                nc.gpsimd.collective_compute(
                    "ReduceScatter", A.add,
                    replica_groups=[[0, 1, 2, 3], [4, 5, 6, 7]],
                    ins=[zi[:]], outs=[zr[:]],
                )
                zo = st3.tile([P, L], bf16, tag="zo")
                nc.sync.dma_start(out=zo[:], in_=zr[:])
                # gate via tanh (same act table as exp):
                #   sigmoid(t) = 0.5*tanh(t/2) + 0.5
                # out+1 = attnc + 0.5*(d + gt*d),  d = repc - attnc
                t3 = st3.tile([P, L], f32, tag="t3")
                nc.vector.tensor_tensor(out=t3[:], in0=zl[:], in1=zo[:], op=A.add)
                gt = st3.tile([P, L], f32, tag="gt")
                nc.scalar.activation(out=gt[:], in_=t3[:], func=F.Tanh, scale=0.5)
                d = st3.tile([P, L], f32, tag="d")
                nc.gpsimd.tensor_tensor(
                    out=d[:], in0=rep_hs, in1=attnT[:], op=A.subtract
                )
                m = st3.tile([P, L], f32, tag="m")
                nc.gpsimd.tensor_tensor(out=m[:], in0=gt[:], in1=d[:], op=A.mult)
                u = st3.tile([P, L], f32, tag="u")
                nc.gpsimd.tensor_tensor(out=u[:], in0=d[:], in1=m[:], op=A.add)
                o = st3.tile([P, L], f32, tag="o")
                nc.vector.scalar_tensor_tensor(
                    out=o[:], in0=u[:], scalar=0.5, in1=attnT[:],
                    op0=A.mult, op1=A.add,
                )
                nc.sync.dma_start(out=out_hsT[:], in_=o[:])

    _CACHE[repeat] = nc
    return nc


def _make_in_maps(inputs):
    import ml_dtypes

    nbf = ml_dtypes.bfloat16
    x = np.asarray(inputs["x"], np.float32)
    fc_w = np.ascontiguousarray(np.asarray(inputs["fc_w"], np.float32))
    fc_b = np.asarray(inputs["fc_b"], np.float32)
    w1_w = np.asarray(inputs["w1_w"], np.float32)
    w1_b = np.asarray(inputs["w1_b"], np.float32)
    w2_w = np.asarray(inputs["w2_w"], np.float32)
    w2_b = np.asarray(inputs["w2_b"], np.float32)
    b_logit = np.asarray(inputs["b_logit"], np.float32)
    wf1_w = np.asarray(inputs["wf1_w"], np.float32)
    wf2_w = np.asarray(inputs["wf2_w"], np.float32)
    bf = np.asarray(inputs["bf"], np.float32)

    # device works with repc = rep+1 and attnc = attn+1; fold the -1
    # shifts into the matmul biases (rep @ W = repc @ W - colsum(W)).
    w1_cs = w1_w.sum(axis=0)
    w2_cs = w2_w.sum(axis=0)
    wf1_cs = wf1_w.sum(axis=0)
    wf2_cs = wf2_w.sum(axis=0)

    in_maps = []
    for c in range(N_CORES):
        b, hs = c // 4, c % 4
        H = slice(P * hs, P * (hs + 1))
        # fcw column order: ht=0 must be the core's own h-slice so that
        # repT[0] is rep_hs; remaining slices fill ht=1..3.
        order = [hs] + [t for t in range(4) if t != hs]
        fcw_perm = np.ascontiguousarray(
            np.concatenate([fc_w[:, P * t:P * (t + 1)] for t in order], axis=1)
        )
        fcb_perm = np.ascontiguousarray(
            np.concatenate([fc_b[P * t:P * (t + 1)] for t in order])
        ).reshape(DH, 1)
        # dep/head/wf1 matmuls contract over the permuted h order, so
        # their weight ROWS are permuted identically.
        rperm = np.concatenate(
            [np.arange(P * t, P * (t + 1)) for t in order]
        )
        in_maps.append({
            "xbt": np.ascontiguousarray(x[b].T),
            "fcw": fcw_perm,
            "fcb": fcb_perm,
            "w12c": np.ascontiguousarray(
                np.concatenate([w1_w[rperm][:, H], w2_w[rperm][:, H]], axis=1)
            ),
            "wf1c": np.ascontiguousarray(wf1_w[rperm][:, H]),
            "wf2r": np.ascontiguousarray(wf2_w[H, :].astype(nbf)),
            "w1bc": np.ascontiguousarray(
                (w1_b - w1_cs)[H].reshape(P, 1)
            ),
            "hbc": np.ascontiguousarray(
                (w2_b + b_logit - w2_cs)[H].reshape(P, 1)
            ),
            "bfc": np.ascontiguousarray(
                (bf - wf1_cs - wf2_cs)[H].reshape(P, 1)
            ),
        })
    return in_maps


def kernel(**inputs):
    from concourse.bass_utils import run_bass_kernel_spmd

    nc = _build()
    in_maps = _make_in_maps(inputs)
    res = run_bass_kernel_spmd(nc, in_maps, core_ids=list(range(N_CORES)))
    out = np.empty((B, L, DH), np.float32)
    for c in range(N_CORES):
        b, hs = c // 4, c % 4
        # device returns out+1 (attnc/repc shift); undo it here
        out[b, :, P * hs:P * (hs + 1)] = res.results[c]["out_hsT"].T - 1.0
    return out
